# revision 1
# baseline (speedup 1.0000x reference)
"""Multi-head causal attention (B=2, T=2048, E=1024, H=16, D=64) on 8 trn2 cores.

Sharding: tensor-parallel over heads — core c owns heads {2c, 2c+1} (a 128-wide
slice of the hidden dim). Each core computes q/k/v projections for its heads
over the full sequence, causal attention, and a partial output projection
(contraction over its 128 rows of Wo). The host sums the 8 partials + bias.

Per-core device program (SPMD — one NEFF, different weight slices per core):
  projections: QT/KT = (W.T @ xT) in [dim, token] layout (weight-stationary,
    token-moving N=512); V in natural [token, dim|1] layout via
    xT-chunk-stationary matmuls. The appended ones column makes the P@V
    matmul emit Z = sum(exp) as psum row 64 for free. Batch-1 projections are
    emitted in small units interleaved into batch-0's attention waves so the
    PE slack there absorbs them.
  attention, per (batch, 512-wide tq chunk), in waves of two 128-row tk
    blocks: S^T = K Q^T with both heads packed on the PE via row tiling
    (contraction rows 0-63 / 64-127 run concurrently), causally trimmed
    moving ranges; P^T = exp(0.125 * S^T) on ScalarE straight out of PSUM
    (safe without max-subtraction: scores ~ N(0,1), |s| < ~7); diagonal
    128x128 triangle masks multiplied into P^T on VectorE; O^T|Z = [V|1]^T
    P^T with causality-limited moving ranges; 1/Z (fp16) broadcast across
    the 64 head dims by a K=1 outer-product matmul; normalize on VectorE.
  output: out[tq, :] = O^T.T @ Wo_slice (K=128), copies + DMA per 128 rows.

Timing signal during development was concourse's TimelineSim cost model
(no NTFF profiling exists under this axon client); measured 164.4us per core,
engine busy: PE 118us, ScalarE 95us, VectorE 79us, DMA 73us. Weight DMAs are
queue-ordered so the first projection matmul's gates (wq + first xT pair)
land ahead of weights not needed until later.
"""

import os
import numpy as np
import ml_dtypes

import concourse.bass as bass
import concourse.tile as tile
from concourse import bacc, mybir
from concourse.bass_utils import run_bass_kernel_spmd
from contextlib import ExitStack

B, T, E, H, D = 2, 2048, 1024, 16, 64
BT = B * T            # 4096 tokens total
NCORE = 8
KC = E // 128         # contraction chunks for projections = 8
CQ = 512              # tq chunk width
NQB = T // CQ         # tq chunks per batch = 4
NKB = T // 128        # tk blocks per batch = 16

F32 = mybir.dt.float32
BF16 = mybir.dt.bfloat16
AF = mybir.ActivationFunctionType

_cache = {}


def _build():
    nc = bacc.Bacc("TRN2", target_bir_lowering=False, debug=False,
                   num_devices=NCORE)

    xT = nc.dram_tensor("xT", [E, BT], BF16, kind="ExternalInput").ap()
    wq = nc.dram_tensor("wq", [128, E], BF16, kind="ExternalInput").ap()
    wk = nc.dram_tensor("wk", [128, E], BF16, kind="ExternalInput").ap()
    wv = nc.dram_tensor("wv", [128, E], BF16, kind="ExternalInput").ap()
    wo = nc.dram_tensor("wo", [128, E], BF16, kind="ExternalInput").ap()
    tri = nc.dram_tensor("tri", [128, 128], BF16, kind="ExternalInput").ap()
    out = nc.dram_tensor("out", [BT, E], F32, kind="ExternalOutput").ap()

    with tile.TileContext(nc) as tc, ExitStack() as ctx:
        pers = ctx.enter_context(tc.tile_pool(name="pers", bufs=1))

        wq_sb = pers.tile([128, KC, 128], BF16, tag="wq")
        wk_sb = pers.tile([128, KC, 128], BF16, tag="wk")
        wv_sb = pers.tile([128, KC, 128], BF16, tag="wv")
        wo_sb = pers.tile([128, E], BF16, tag="wo")
        tri_sb = pers.tile([128, 128], BF16, tag="tri")
        ones_sb = pers.tile([128, 64], mybir.dt.float16, tag="ones")
        qt_sb = pers.tile([128, BT], BF16, tag="qt")    # [dims(2 heads), tok]
        kt_sb = pers.tile([128, BT], BF16, tag="kt")
        # V natural + ones col per head: [tok%128, blk, h, d|1]
        v_sb = pers.tile([128, BT // 128, 2, 65], BF16, tag="v")
        ot_sb = pers.tile([128, BT], BF16, tag="ot")    # attn out, [dims, tok]

        # wq/wk queued first on HWDGE; wv/tri/wo go after the first xT pair
        # (they are not needed until ~10us in) so the first projection
        # matmuls start as early as possible.
        nc.sync.dma_start(wq_sb[:], wq.rearrange("p (kc d) -> p kc d", kc=KC))
        nc.sync.dma_start(wk_sb[:], wk.rearrange("p (kc d) -> p kc d", kc=KC))
        nc.vector.memset(ones_sb[:], 1.0)
        nc.vector.memset(v_sb[:, :, :, 64:65], 1.0)

        def load_late_weights():
            nc.sync.dma_start(wv_sb[:],
                              wv.rearrange("p (kc d) -> p kc d", kc=KC))
            nc.sync.dma_start(tri_sb[:], tri[:])
            nc.sync.dma_start(wo_sb[:], wo[:])

        # Unified pools: PSUM tags share the 8 banks so batch-1 projections
        # overlap batch-0 attention, and attention waves start as soon as
        # their tk blocks are projected.
        xts_pool = ctx.enter_context(tc.tile_pool(name="xts", bufs=16))
        sc_pool = ctx.enter_context(tc.tile_pool(name="sc", bufs=2, space="PSUM"))
        pv_pool = ctx.enter_context(tc.tile_pool(name="pv", bufs=2, space="PSUM"))
        vps = ctx.enter_context(tc.tile_pool(name="vps", bufs=1, space="PSUM"))
        ops_pool = ctx.enter_context(tc.tile_pool(name="ops", bufs=1, space="PSUM"))
        pt_pool = ctx.enter_context(tc.tile_pool(name="pt", bufs=3))
        zr_pool = ctx.enter_context(tc.tile_pool(name="zr", bufs=3))
        zbs_pool = ctx.enter_context(tc.tile_pool(name="zbs", bufs=3))
        ost_pool = ctx.enter_context(tc.tile_pool(name="ost", bufs=6))

        def proj_pair_units(t0):
                xts = []
                for kc in range(KC):
                    xt = xts_pool.tile([128, 2 * CQ], BF16, tag="xt",
                                       name=f"xt_{t0}_{kc}")
                    nc.sync.dma_start(
                        xt[:], xT[kc * 128:(kc + 1) * 128,
                                  t0 * CQ:(t0 + 2) * CQ])
                    xts.append(xt)

                def qk_unit(w_sb, dst_sb, hf):
                    t_ = t0 + hf
                    def emit():
                        ps = sc_pool.tile([128, CQ], F32, tag="sc",
                                          name=f"qkps{t_}_{id(w_sb)}")
                        for kc in range(KC):
                            nc.tensor.matmul(
                                ps[:], w_sb[:, kc],
                                xts[kc][:, hf * CQ:(hf + 1) * CQ],
                                start=(kc == 0), stop=(kc == KC - 1))
                        if t_ < 4:
                            nc.scalar.copy(
                                dst_sb[:, t_ * CQ:(t_ + 1) * CQ], ps[:])
                        else:
                            nc.vector.tensor_copy(
                                dst_sb[:, t_ * CQ:(t_ + 1) * CQ], ps[:])
                    return emit

                def v_unit(hf):
                    t_ = t0 + hf
                    def emit():
                        v_ps = vps.tile([128, CQ], F32, tag="v",
                                        name=f"vps{t_}")
                        for j in range(CQ // 128):
                            jf = hf * CQ + j * 128
                            for kc in range(KC):
                                nc.tensor.matmul(
                                    v_ps[:, j * 128:(j + 1) * 128],
                                    xts[kc][:, jf:jf + 128],
                                    wv_sb[:, kc], start=(kc == 0),
                                    stop=(kc == KC - 1))
                        b4 = t_ * (CQ // 128)
                        nc.vector.tensor_copy(
                            v_sb[:, b4:b4 + 4, :, 0:64],
                            v_ps[:].rearrange("p (j h v) -> p j h v",
                                              j=4, h=2))
                    return emit

                return [qk_unit(wq_sb, qt_sb, 0), qk_unit(wk_sb, kt_sb, 0),
                        v_unit(0), qk_unit(wq_sb, qt_sb, 1),
                        qk_unit(wk_sb, kt_sb, 1), v_unit(1)]

        first = True
        for t0 in (0, 2):  # batch-0 projections first
            units0 = proj_pair_units(t0)  # emits the pair's xT DMAs
            if first:
                load_late_weights()
                first = False
            for u in units0:
                u()
        b1_units = None
        for b in range(B):
            if True:
                tb = b * T  # token offset of this batch
                for icq, cq in enumerate(range(NQB) if b == 0
                                         else range(NQB - 1, -1, -1)):
                    if b == 0 and icq in (0, 2):
                        b1_units = proj_pair_units(T // CQ + icq)
                    units = (b1_units[(icq % 2) * 3:(icq % 2) * 3 + 3]
                             if b == 0 else [])
                    tq0 = cq * CQ
                    nblk = (tq0 + CQ) // 128  # causal: tk blocks needed
                    pt = [pt_pool.tile([128, NKB, CQ], BF16, tag=f"pt{h}",
                                       name=f"pt{h}_{b}_{cq}")
                          for h in range(2)]
                    pv = [pv_pool.tile([128, CQ], F32, tag="pv",
                                       name=f"pv{h}_{b}_{cq}")
                          for h in range(2)]

                    for w in range(nblk // 2):  # waves of 2 tk blocks
                        kbs = (2 * w, 2 * w + 1)
                        sc = [sc_pool.tile([128, 2 * CQ], F32, tag="sc",
                                           name=f"sc{h}_{b}_{cq}_{w}")
                              for h in range(2)]
                        for i, kb in enumerate(kbs):
                            tk0 = kb * 128
                            f0 = max(tk0 - tq0, 0)
                            for h in range(2):
                                hs = slice(h * 64, (h + 1) * 64)
                                nc.tensor.matmul(
                                    sc[h][:, i * CQ + f0:(i + 1) * CQ],
                                    kt_sb[hs, tb + tk0:tb + tk0 + 128],
                                    qt_sb[hs, tb + tq0 + f0:tb + tq0 + CQ],
                                    start=True, stop=True)
                        for h in range(2):
                            nc.scalar.activation(
                                pt[h][:, 2 * w:2 * w + 2], sc[h][:],
                                AF.Exp, scale=float(D) ** -0.5)
                        for i, kb in enumerate(kbs):
                            tk0 = kb * 128
                            s = tk0 - tq0
                            f0 = max(s, 0)  # first causally-valid tq col
                            for h in range(2):
                                if 0 <= s < CQ:  # diagonal: triangle mask
                                    m_eng = (nc.gpsimd if (b == 1 and icq >= 2)
                                             else nc.vector)
                                    m_eng.tensor_mul(
                                        pt[h][:, kb, s:s + 128],
                                        pt[h][:, kb, s:s + 128], tri_sb[:])
                                nc.tensor.matmul(
                                    pv[h][0:65, f0:CQ],
                                    v_sb[:, b * NKB + kb, h],
                                    pt[h][:, kb, f0:CQ],
                                    start=(kb == 0), stop=(kb == nblk - 1))

                        if units:
                            units.pop(0)()
                    for u in units:
                        u()
                    units = []

                    # normalize: zr = 1/Z; zb = ones x zr (broadcast); mul
                    zr = zr_pool.tile([128, 2 * CQ], mybir.dt.float16, tag="zr",
                                      name=f"zr_{b}_{cq}")
                    zb_ps = ops_pool.tile([128, CQ], F32, tag="o",
                                          name=f"zb_{b}_{cq}")
                    zb_sb = zbs_pool.tile([128, CQ], F32, tag="zbs",
                                          name=f"zbs_{b}_{cq}")
                    for h in range(2):
                        zrh = zr[64:65, h * CQ:(h + 1) * CQ]
                        with nc.allow_low_precision(
                                reason="1/Z in fp16 (2.4e-4 rel) feeds the "
                                       "K=1 broadcast matmul at full PE rate"):
                            nc.vector.reciprocal(zrh, pv[h][64:65, :])
                        nc.tensor.matmul(
                            zb_ps[h * 64:(h + 1) * 64, :],
                            ones_sb[64:65, :], zrh, start=True, stop=True)
                    if b == 1 and icq >= 2:
                        nc.scalar.copy(zb_sb[:], zb_ps[:])
                    else:
                        nc.vector.tensor_copy(zb_sb[:], zb_ps[:])
                    for h in range(2):
                        nc.vector.tensor_mul(
                            ot_sb[h * 64:(h + 1) * 64, tb + tq0:tb + tq0 + CQ],
                            pv[h][0:64, :], zb_sb[h * 64:(h + 1) * 64, :])

                    # phase 3 for this chunk
                    for j in range(CQ // 128):
                        tqg = tb + tq0 + j * 128
                        ost = ost_pool.tile([128, 1024], F32, tag="ost",
                                            name=f"ost_{b}_{cq}_{j}")
                        for eh in range(2):
                            fin_pool, fin_tag = ((sc_pool, "sc")
                                                 if b == 1 and icq == 3
                                                 else (ops_pool, "o"))
                            o_ps = fin_pool.tile(
                                [128, 512], F32, tag=fin_tag,
                                name=f"o_{b}_{cq}_{j}_{eh}")
                            nc.tensor.matmul(
                                o_ps[:], ot_sb[:, tqg:tqg + 128],
                                wo_sb[:, eh * 512:(eh + 1) * 512],
                                start=True, stop=True)
                            dst = ost[:, eh * 512:(eh + 1) * 512]
                            if b == 1 and icq >= 2 and eh == 1:
                                nc.scalar.copy(dst, o_ps[:])
                            else:
                                nc.vector.tensor_copy(dst, o_ps[:])
                        nc.sync.dma_start(out[tqg:tqg + 128, :], ost[:])

    nc.compile()
    return nc


def _host_prep(x, Wq, Wk, Wv, Wo):
    bf = ml_dtypes.bfloat16
    xT = np.ascontiguousarray(
        np.asarray(x, dtype=np.float32).reshape(BT, E).T).astype(bf)

    # tri[p, f] = 1 where kept (f >= p), applied to the diagonal 128x128
    # sub-block of P^T (tk on partitions, tq on free)
    p = np.arange(128)[:, None]
    f = np.arange(128)[None, :]
    tri = (f >= p).astype(bf)

    def perm(w):
        # [E, 128] -> [128p, kc, 128d] flattened: w[kc*128+p, d] -> out[p, kc, d]
        return np.ascontiguousarray(
            w.reshape(KC, 128, 128).transpose(1, 0, 2).reshape(128, E)).astype(bf)

    Wq = np.asarray(Wq, dtype=np.float32)
    Wk = np.asarray(Wk, dtype=np.float32)
    Wv = np.asarray(Wv, dtype=np.float32)
    Wo = np.asarray(Wo, dtype=np.float32)

    in_maps = []
    for c in range(NCORE):
        sl = slice(c * 128, (c + 1) * 128)
        in_maps.append({
            "xT": xT,
            "wq": perm(Wq[:, sl]),
            "wk": perm(Wk[:, sl]),
            "wv": perm(Wv[:, sl]),
            "wo": np.ascontiguousarray(Wo[sl, :]).astype(bf),
            "tri": tri,
        })
    return in_maps


def kernel(x, Wq, Wk, Wv, Wo, bo, _trace=False, _trace_kwargs=None):
    if "nc" not in _cache:
        _cache["nc"] = _build()
    nc = _cache["nc"]

    in_maps = _host_prep(x, Wq, Wk, Wv, Wo)
    kw = {}
    if _trace:
        kw = dict(trace=True, trace_cores=[0], **(_trace_kwargs or {}))
    res = run_bass_kernel_spmd(nc, in_maps, core_ids=list(range(NCORE)), **kw)
    _cache["last_result"] = res

    total = np.zeros((BT, E), dtype=np.float32)
    for r in res.results:
        total += r["out"]
    total += np.asarray(bo, dtype=np.float32)[None, :]
    return total.reshape(B, T, E)

